# revision 25
# baseline (speedup 1.0000x reference)
import sys

if "/opt/trn_rl_repo" not in sys.path:
    sys.path.insert(0, "/opt/trn_rl_repo")

import numpy as np
import ml_dtypes

import concourse.bass as bass
import concourse.tile as tile
from concourse import bacc, mybir
from concourse.bass_utils import run_bass_kernel_spmd
from concourse.masks import make_upper_triangular

F32 = mybir.dt.float32
F32R = mybir.dt.float32r
BF16 = mybir.dt.bfloat16
BF16NP = ml_dtypes.bfloat16

# Problem shape (hardcoded per contract)
B, T, D = 4, 2048, 768
H, HD = 12, 64
N_CORES = 8
HEADS_PER_CORE = 6          # 12 heads / 2 groups
CPC = HEADS_PER_CORE * HD   # 384 qkv columns per core
TC = T // 128               # 16 token tiles of 128
DC = D // 128               # 6 chunks of the model dim
CC = CPC // 128             # 3 chunks of this core's head cols
OC = D // 128               # 6 output-col chunks
WT = 512                    # PSUM f32 bank width
TW = T // WT                # 4 wide token tiles
SCW = 1024                  # score chunk width (bf16 moving operand max)

# ragged P^T layout: block jc covers i in [jc*128, T); OFF[jc] is its col offset
OFF = [0] * (TC + 1)
for _jc in range(TC):
    OFF[_jc + 1] = OFF[_jc] + (TC - _jc) * 128
PT_COLS = OFF[TC]  # 17408

_CACHE = {}
CONFIG = {"bf16_inputs": True, "fast_recip": True}


def _build_nc(bf16_inputs=True, fast_recip=True):
    IDT = BF16 if bf16_inputs else F32R
    nc = bacc.Bacc("TRN2", target_bir_lowering=False, debug=False)

    xT = nc.dram_tensor("xT", [D, T], IDT, kind="ExternalInput")
    wq = nc.dram_tensor("wq", [D, CPC], IDT, kind="ExternalInput")
    wk = nc.dram_tensor("wk", [D, CPC], IDT, kind="ExternalInput")
    wv = nc.dram_tensor("wv", [D, CPC], IDT, kind="ExternalInput")
    bq = nc.dram_tensor("bq", [128, CC], F32, kind="ExternalInput")
    bk = nc.dram_tensor("bk", [128, CC], F32, kind="ExternalInput")
    bv = nc.dram_tensor("bv", [128, CPC], F32, kind="ExternalInput")
    wo = nc.dram_tensor("wo", [CPC, D], IDT, kind="ExternalInput")
    yT = nc.dram_tensor("yT", [D, T], F32, kind="ExternalOutput")

    with tile.TileContext(nc) as tc:
        with tc.tile_pool(name="persist", bufs=1) as pp:
            WDT = BF16 if bf16_inputs else F32R
            qT_sb = pp.tile([128, CC, T], BF16)     # q^T, head cols on partitions
            # k^T stored per head with the partner half zeroed: K=128 matmuls
            # keep the PE activity monitor warm (K=64 runs at half clock)
            kTz_sb = pp.tile([128, HEADS_PER_CORE, T], BF16)
            # v with denominator-ones column, positioned so that the PV output
            # lands on the partitions the head's attnT half needs directly:
            #   even head: [v(0:64) | ones@64 | 0...]  -> out rows 0:64 + l@64
            #   odd head:  [0(:32) | ones@32 | 0 | v(64:128)] -> l@32 + rows 64:128
            v_sb = pp.tile([128, TC, HEADS_PER_CORE, 128], BF16)
            attnT_sb = pp.tile([128, CC, T], WDT)  # attention out, [cols, T]
            wo_sb = pp.tile([128, CC, D], WDT)
            bq_sb = pp.tile([128, CC], F32)
            bk_sb = pp.tile([128, CC], F32)
            bv_sb = pp.tile([128, HEADS_PER_CORE, HD], F32)
            maskf = pp.tile([128, 128], F32)
            mask01 = pp.tile([128, 128], BF16)
            ones_sb = pp.tile([128, 128], BF16)  # lhsT for denominator broadcast

            nc.sync.dma_start(bq_sb[:], bq.ap())
            nc.sync.dma_start(bk_sb[:], bk.ap())
            nc.sync.dma_start(bv_sb[:], bv.ap())
            # mask01[j, i] = 1.0 if j <= i else 0.0 (valid causal region, S^T coords)
            make_upper_triangular(nc, maskf, val=1.0, diag=True)
            nc.vector.tensor_copy(mask01[:], maskf[:])
            nc.vector.memzero(kTz_sb[:])
            nc.gpsimd.memset(ones_sb[:], 1.0)
            nc.vector.memzero(v_sb[:])
            nc.gpsimd.memset(v_sb[:, :, 0:HEADS_PER_CORE:2, HD : HD + 1], 1.0)
            nc.gpsimd.memset(v_sb[:, :, 1:HEADS_PER_CORE:2, 32:33], 1.0)

            # ---------------- Phase A: qkv projection ----------------
            with (
                tc.tile_pool(name="loadA", bufs=1) as pA,
                tc.tile_pool(name="psumA", bufs=2, space="PSUM") as psA,
            ):
                xT_sb = pA.tile([128, DC, T], WDT)
                wq_sb = pA.tile([128, DC, CPC], WDT)
                wk_sb = pA.tile([128, DC, CPC], WDT)
                wv_sb = pA.tile([128, DC, CPC], WDT)
                xT_r = xT.ap().rearrange("(o p) t -> p o t", p=128)
                wq_r = wq.ap().rearrange("(o p) c -> p o c", p=128)
                wk_r = wk.ap().rearrange("(o p) c -> p o c", p=128)
                wv_r = wv.ap().rearrange("(o p) c -> p o c", p=128)
                # need-order: first token block + q/k weights first, W_o last
                nc.sync.dma_start(xT_sb[:, :, 0:WT], xT_r[:, :, 0:WT])
                for di in range(DC):
                    nc.sync.dma_start(wq_sb[:, di], wq_r[:, di])
                    nc.sync.dma_start(wk_sb[:, di], wk_r[:, di])
                for di in range(DC):
                    nc.sync.dma_start(wv_sb[:, di], wv_r[:, di])
                for tw in range(1, TW):
                    sp = slice(tw * WT, (tw + 1) * WT)
                    nc.sync.dma_start(xT_sb[:, :, sp], xT_r[:, :, sp])
                nc.sync.dma_start(
                    wo_sb[:], wo.ap().rearrange("(c p) o -> p c o", p=128)
                )

                for tw in range(TW):
                    sp = slice(tw * WT, (tw + 1) * WT)
                    for hc in range(CC):
                        cs = slice(hc * 128, (hc + 1) * 128)
                        ps_q = psA.tile([128, WT], F32, tag="pqk", bufs=3)
                        for di in range(DC):
                            nc.tensor.matmul(
                                ps_q[:],
                                wq_sb[:, di, cs],
                                xT_sb[:, di, sp],
                                start=(di == 0),
                                stop=(di == DC - 1),
                            )
                        nc.vector.tensor_scalar_add(
                            qT_sb[:, hc, sp], ps_q[:], bq_sb[:, hc : hc + 1]
                        )
                        ps_k = psA.tile([128, WT], F32, tag="pqk", bufs=3)
                        for di in range(DC):
                            nc.tensor.matmul(
                                ps_k[:],
                                wk_sb[:, di, cs],
                                xT_sb[:, di, sp],
                                start=(di == 0),
                                stop=(di == DC - 1),
                            )
                        nc.vector.tensor_scalar_add(
                            kTz_sb[0:HD, 2 * hc, sp],
                            ps_k[0:HD, :],
                            bk_sb[0:HD, hc : hc + 1],
                        )
                        nc.vector.tensor_scalar_add(
                            kTz_sb[HD:, 2 * hc + 1, sp],
                            ps_k[HD:, :],
                            bk_sb[HD:, hc : hc + 1],
                        )
                    for t4 in range(4):
                        tj = tw * 4 + t4
                        ps_v = psA.tile(
                            [128, HEADS_PER_CORE, HD], F32, tag="psv", bufs=2
                        )
                        for di in range(DC):
                            nc.tensor.matmul(
                                ps_v[:],
                                xT_sb[:, di, tj * 128 : (tj + 1) * 128],
                                wv_sb[:, di, :],
                                start=(di == 0),
                                stop=(di == DC - 1),
                            )
                        # even heads: data cols 0:64; odd heads: data cols 64:128
                        nc.vector.tensor_add(
                            v_sb[:, tj, 0:HEADS_PER_CORE:2, 0:HD],
                            ps_v[:, 0:HEADS_PER_CORE:2, :],
                            bv_sb[:, 0:HEADS_PER_CORE:2, :],
                        )
                        nc.vector.tensor_add(
                            v_sb[:, tj, 1:HEADS_PER_CORE:2, HD:128],
                            ps_v[:, 1:HEADS_PER_CORE:2, :],
                            bv_sb[:, 1:HEADS_PER_CORE:2, :],
                        )

            # ---------- Phase B: causal attention, big-N formulation ----------
            with (
                tc.tile_pool(name="pB", bufs=2) as pB,
                tc.tile_pool(name="outp", bufs=2) as outp,
                tc.tile_pool(name="psumB", bufs=3, space="PSUM") as psB,
            ):
                # one persistent PSUM mega-tile; score matmuls pack into its
                # four 512-quarters as a manual ring (range-based deps give
                # exactly the double-buffering we need), so every exp chunk
                # is a full 1024 wide — 17 ACT instructions per head, not 28
                st_mega = psB.tile([128, 4 * WT], F32, tag="st", bufs=1)

                def score_steps(h, pT):
                    """Closures: one 1024-col exp chunk each (2-3 packed MMs)."""
                    hc = h // 2
                    # pieces: (jc, s0_in_block, n, pT_col)
                    pieces = []
                    pcol = 0
                    for jc in range(TC):
                        w = (TC - jc) * 128
                        s0 = 0
                        while s0 < w:
                            n = min(WT - (pcol % WT), w - s0)
                            pieces.append((jc, s0, n, pcol))
                            s0 += n
                            pcol += n
                    ring0 = (h % 2) * SCW  # alternate mega-tile halves
                    total = pcol
                    chunk = []
                    cbase = 0
                    for p in pieces:
                        chunk.append(p)
                        cend = p[3] + p[2]
                        if cend - cbase == SCW or cend == total:

                            def step(chunk=chunk, cbase=cbase, cend=cend):
                                for jc, s0, n, pc in chunk:
                                    rp = (ring0 + pc) % (2 * SCW)
                                    nc.tensor.matmul(
                                        st_mega[:, rp : rp + n],
                                        kTz_sb[:, h, jc * 128 : (jc + 1) * 128],
                                        qT_sb[
                                            :,
                                            hc,
                                            jc * 128 + s0 : jc * 128 + s0 + n,
                                        ],
                                        start=True,
                                        stop=True,
                                    )
                                rb = (ring0 + cbase) % (2 * SCW)
                                nc.scalar.activation(
                                    pT[:, cbase:cend],
                                    st_mega[:, rb : rb + (cend - cbase)],
                                    mybir.ActivationFunctionType.Exp,
                                )
                                for jc, s0, n, pc in chunk:
                                    if s0 == 0:  # chunk opens block jc: mask diag
                                        nc.gpsimd.tensor_mul(
                                            pT[:, OFF[jc] : OFF[jc] + 128],
                                            pT[:, OFF[jc] : OFF[jc] + 128],
                                            mask01[:],
                                        )

                            yield step
                            chunk = []
                            cbase = cend

                def d_steps(tj):
                    """W_o matmuls for one completed 512-wide token block.
                    Reuses the score mega-tile quarters (scores are done)."""
                    for oc in range(OC):

                        def step(oc=oc, tj=tj):
                            rp = ((tj * OC + oc) % 4) * WT
                            ps_wo = st_mega[:, rp : rp + WT]
                            for dc in range(CC):
                                nc.tensor.matmul(
                                    ps_wo,
                                    wo_sb[:, dc, oc * 128 : (oc + 1) * 128],
                                    attnT_sb[:, dc, tj * WT : (tj + 1) * WT],
                                    start=(dc == 0),
                                    stop=(dc == CC - 1),
                                )
                            ot = outp.tile([128, WT], F32, tag="ot")
                            nc.scalar.copy(ot[:], ps_wo)
                            nc.sync.dma_start(
                                yT.ap()[
                                    oc * 128 : (oc + 1) * 128, tj * WT : (tj + 1) * WT
                                ],
                                ot[:],
                            )

                        yield step

                def pv_steps(h, pT, emit_wo):
                    """PV in transposed form; div chain lags one quarter; the
                    last head's completion enqueues W_o work per block."""
                    hc = h // 2
                    odd = h % 2 == 1
                    lrow = 32 if odd else HD    # partition holding l
                    mhi = 128 if odd else HD + 1  # lhsT col span (from 0)
                    oTs, oUs, lPs, rcbs = {}, {}, {}, {}

                    def div_chain(q):
                        i0 = q * WT

                        def c_copy(q=q):
                            oU = pB.tile(
                                [128, WT], BF16, tag="oU", bufs=2,
                                name=f"oU{h}_{q}",
                            )
                            oUs[q] = oU
                            if odd:
                                nc.vector.tensor_copy(oU[HD:128, :], oTs[q][HD:128, :])
                                nc.vector.tensor_copy(oU[32:33, :], oTs[q][32:33, :])
                            else:
                                nc.vector.tensor_copy(oU[0:HD, :], oTs[q][0:HD, :])
                                nc.vector.tensor_copy(
                                    oU[HD : HD + 1, :], oTs[q][HD : HD + 1, :]
                                )

                        def c_bcast(q=q):
                            # broadcast l to ALL 128 partitions so the custom
                            # DVE reciprocal runs at partition base 0 (it
                            # mishandles non-zero partition offsets)
                            lP = psB.tile(
                                [128, WT], F32, tag="lP", bufs=1, name=f"lP{h}_{q}"
                            )
                            lPs[q] = lP
                            nc.tensor.matmul(
                                lP[:, :],
                                ones_sb[lrow : lrow + 1, :],
                                oUs[q][lrow : lrow + 1, :],
                                start=True,
                                stop=True,
                            )

                        def c_recip(q=q):
                            rcb = pB.tile(
                                [128, WT], F32, tag="rcb", bufs=2, name=f"rcb{h}_{q}"
                            )
                            rcbs[q] = rcb
                            if fast_recip:
                                nc.vector.reciprocal_approx_fast(
                                    rcb[:, :], lPs[q][:, :]
                                )
                            else:
                                nc.vector.reciprocal(
                                    rcb[:, :], lPs[q][:, :]
                                )

                        def c_mul(q=q):
                            i0 = q * WT
                            ob = HD if odd else 0
                            nc.vector.tensor_mul(
                                attnT_sb[ob : ob + HD, hc, i0 : i0 + WT],
                                oUs[q][ob : ob + HD, :],
                                rcbs[q][ob : ob + HD, :],
                            )

                        return [c_copy, c_bcast, c_recip, c_mul]

                    for q in range(TW):
                        i0 = q * WT
                        jhi = min(4 * q + 3, TC - 1)
                        jcs = list(range(jhi + 1))
                        # chunk the accumulation into groups of <=4 matmuls
                        for g0 in range(0, len(jcs), 4):
                            grp = jcs[g0 : g0 + 4]

                            def step(q=q, i0=i0, jhi=jhi, grp=grp, g0=g0):
                                if g0 == 0:
                                    oTs[q] = psB.tile(
                                        [128, WT], F32, tag="oT", bufs=3,
                                        name=f"oT{h}_{q}",
                                    )
                                oT = oTs[q]
                                for jc in grp:
                                    lo = max(jc * 128, i0)
                                    rhs = pT[
                                        :,
                                        OFF[jc] + lo - jc * 128 : OFF[jc]
                                        + i0
                                        + WT
                                        - jc * 128,
                                    ]
                                    nc.tensor.matmul(
                                        oT[0:mhi, lo - i0 : WT],
                                        v_sb[:, jc, h, 0:mhi],
                                        rhs,
                                        start=(jc == 0),
                                        stop=(jc == jhi),
                                    )

                            yield step
                        if emit_wo:
                            # lag-0 divide + W_o interleave: keeps the PE fed
                            # with W_o matmuls while the DVE chain runs, and
                            # minimizes the serial tail after the last quarter
                            cc, cb, cr, cm = div_chain(q)
                            yield cc
                            if q >= 1:
                                yield from d_steps(q - 1)
                            yield cb
                            yield cr
                            yield cm
                            if q == TW - 1:
                                yield from d_steps(q)
                        elif q >= 1:
                            for c in div_chain(q - 1):
                                yield c
                    if not emit_wo:
                        for c in div_chain(TW - 1):
                            yield c

                def interleave(a_steps, b_steps):
                    """Emit steps from both lists, spreading b evenly among a."""
                    a, b = list(a_steps), list(b_steps)
                    if not b:
                        for s in a:
                            s()
                        return
                    ratio = max(1, len(a) // len(b))
                    bi = 0
                    for idx, s in enumerate(a):
                        s()
                        if idx % ratio == ratio - 1 and bi < len(b):
                            b[bi]()
                            bi += 1
                    while bi < len(b):
                        b[bi]()
                        bi += 1

                pTs = {}
                pTs[0] = pB.tile([128, PT_COLS], BF16, tag="pT", name="pT0")
                for s in score_steps(0, pTs[0]):
                    s()
                for h in range(1, HEADS_PER_CORE):
                    pTs[h] = pB.tile([128, PT_COLS], BF16, tag="pT", name=f"pT{h}")
                    interleave(
                        score_steps(h, pTs[h]),
                        pv_steps(h - 1, pTs[h - 1], False),
                    )
                    del pTs[h - 1]
                last = HEADS_PER_CORE - 1
                for s in pv_steps(last, pTs[last], emit_wo=True):
                    s()

    nc.compile()
    return nc


def _get_nc():
    key = ("nc", CONFIG["bf16_inputs"], CONFIG["fast_recip"])
    if key not in _CACHE:
        _CACHE[key] = _build_nc(CONFIG["bf16_inputs"], CONFIG["fast_recip"])
    return _CACHE[key]


def kernel(x, W_qkv, b_qkv, W_o, b_o, **run_kwargs):
    x = np.asarray(x, dtype=np.float32)
    W_qkv = np.asarray(W_qkv, dtype=np.float32)
    b_qkv = np.asarray(b_qkv, dtype=np.float32)
    W_o = np.asarray(W_o, dtype=np.float32)
    b_o = np.asarray(b_o, dtype=np.float32)

    scale = np.float32(1.0) / np.sqrt(np.float32(HD)).astype(np.float32)

    in_maps = []
    for c in range(N_CORES):
        b = c // 2
        g = c % 2
        cs = g * CPC
        q_sl = slice(cs, cs + CPC)
        k_sl = slice(D + cs, D + cs + CPC)
        v_sl = slice(2 * D + cs, 2 * D + cs + CPC)
        idt = BF16NP if CONFIG["bf16_inputs"] else np.float32
        in_maps.append(
            {
                "xT": np.ascontiguousarray(x[b].T).astype(idt),
                "wq": (np.ascontiguousarray(W_qkv[:, q_sl]) * scale).astype(idt),
                "wk": np.ascontiguousarray(W_qkv[:, k_sl]).astype(idt),
                "wv": np.ascontiguousarray(W_qkv[:, v_sl]).astype(idt),
                "bq": np.ascontiguousarray((b_qkv[q_sl] * scale).reshape(CC, 128).T),
                "bk": np.ascontiguousarray(b_qkv[k_sl].reshape(CC, 128).T),
                "bv": np.ascontiguousarray(np.broadcast_to(b_qkv[v_sl], (128, CPC))),
                "wo": np.ascontiguousarray(W_o[cs : cs + CPC, :]).astype(idt),
            }
        )

    nc = _get_nc()
    res = run_bass_kernel_spmd(nc, in_maps, core_ids=list(range(N_CORES)), **run_kwargs)
    _CACHE["last_result"] = res

    out = np.empty((B, T, D), dtype=np.float32)
    for b in range(B):
        acc = res.results[2 * b]["yT"] + res.results[2 * b + 1]["yT"]
        out[b] = acc.T + b_o
    return out
